# revision 24
# baseline (speedup 1.0000x reference)
"""Trainium2 Bass kernel for GQA attention block (nn_Attention_81372450390110).

Module: y = AttnOut(x) with q/k RMSNorm + interleaved RoPE + causal GQA
(NH=16 q heads, KVH=4 kv heads, HD=128, D=2048, B=2, S=2048).

Sharding: 8 cores = 2 batches x 4 KV groups. Core c handles batch c//4 and
KV group c%4 (4 q heads + 1 kv head). Each core computes a full [S, D]
partial of the output projection (row-parallel over heads); the host sums
the 4 group-partials per batch (fp16 partials, fp32 accumulate).

v2 layout strategy (vs v1):
  - transposed PV: stationary = v block (reused across heads/q-tiles),
    streaming p columns -> attT [hd, q] lands directly in PSUM; kills the
    LDWEIGHTS-bound 129-col PV matmuls and all PE transposes
  - softmax denominator l via DVE bf16 accumulation of p tiles + one
    ones-matmul per (head, q-tile) + ones-broadcast matmul for 1/l
  - phase interleaving: attention for q-tile qt emitted right after
    projection block nb=qt+1; o-projection of qt interleaved into
    attention of qt+2; single shared PSUM ring
  - V computed transposed (512-wide streams) then moved to token-major
    via DMA XBAR transpose (free wrt engines)
  - rope half-swap via DVE partition-offset reads (no SBUF-SBUF DMA)
  - DMA schedule: latency-critical loads on the two HWDGE queues
    (sync+scalar) in consumption order; wv/wo on the gpsimd SW queue
  - y output in fp16, written per 128-token stripe
"""

import os
import sys

sys.path.insert(0, "/opt/trn_rl_repo")

import numpy as np
import ml_dtypes

BF16 = ml_dtypes.bfloat16

B = 2
S = 2048
D = 2048
NH = 16
KVH = 4
HD = 128
THETA = 10000.0
EPS = 1e-6
NHL = NH // KVH  # q heads per core (4)
SCALE = 1.0 / float(np.sqrt(HD))

_CACHED = {}


def build_nc(s=S, d=D, nhl=NHL, hd=HD):
    import concourse.mybir as mybir
    import concourse.tile as tile
    from concourse import bacc

    f32 = mybir.dt.float32
    f16 = mybir.dt.float16
    bf16 = mybir.dt.bfloat16
    AF = mybir.ActivationFunctionType

    kc_n = d // 128          # contraction chunks for projections
    nb_n = s // 512          # 512-token blocks
    qt_n = s // 512          # q tiles (512 wide) in attention
    kb_n = s // 128          # k blocks (128 wide)

    nc = bacc.Bacc("TRN2", target_bir_lowering=False, debug=False)

    xT_d = nc.dram_tensor("xT", (d, s), bf16, kind="ExternalInput")
    wq_d = nc.dram_tensor("wq", (d, nhl * hd), bf16, kind="ExternalInput")
    wk_d = nc.dram_tensor("wk", (d, hd), bf16, kind="ExternalInput")
    wv_d = nc.dram_tensor("wv", (d, hd), bf16, kind="ExternalInput")
    wo_d = nc.dram_tensor("wo", (nhl * hd, d), bf16, kind="ExternalInput")
    m1q_d = nc.dram_tensor("m1q", (hd, s), bf16, kind="ExternalInput")
    m2q_d = nc.dram_tensor("m2q", (hd, s), bf16, kind="ExternalInput")
    m1k_d = nc.dram_tensor("m1k", (hd, s), bf16, kind="ExternalInput")
    m2k_d = nc.dram_tensor("m2k", (hd, s), bf16, kind="ExternalInput")
    tri_d = nc.dram_tensor("tri", (128, 128), bf16, kind="ExternalInput")
    y_d = nc.dram_tensor("y", (s, d), f16, kind="ExternalOutput")

    with tile.TileContext(nc) as tc, nc.allow_low_precision(
        reason="bf16 compute by design; fp32 accumulation in PSUM"
    ):
        with (
            tc.tile_pool(name="const", bufs=1) as const,
            tc.tile_pool(name="persist", bufs=1) as persist,
            tc.tile_pool(name="xtp", bufs=2) as xtp,
            tc.tile_pool(name="wa", bufs=3) as wa,
            tc.tile_pool(name="pb", bufs=4) as pb,
            tc.tile_pool(name="lac", bufs=2) as lac,
            tc.tile_pool(name="ysb", bufs=2) as ysbp,
            tc.tile_pool(name="big", bufs=4, space="PSUM") as big,
            tc.tile_pool(name="attp", bufs=2, space="PSUM") as attp,
            tc.tile_pool(name="vps", bufs=1, space="PSUM") as vps,
            tc.tile_pool(name="ssl", bufs=1, space="PSUM") as ssl,
        ):
            # ---- resident weights / coefficients -------------------------
            wq_sb = persist.tile([128, kc_n, nhl * hd], bf16, tag="wq")
            wq_re = wq_d.rearrange("(kc p) m -> p kc m", p=128)
            wk_sb = persist.tile([128, kc_n, hd], bf16, tag="wk")
            wk_re = wk_d.rearrange("(kc p) m -> p kc m", p=128)
            wv_sb = persist.tile([128, kc_n, hd], bf16, tag="wv")
            wv_re = wv_d.rearrange("(kc p) m -> p kc m", p=128)
            wo_sb = persist.tile([128, nhl, d], bf16, tag="wo")
            wo_re = wo_d.rearrange("(h p) m -> p h m", p=128)

            m1q_sb = persist.tile([hd, s], bf16, tag="m1q")
            m2q_sb = persist.tile([hd, s], bf16, tag="m2q")
            m1k_sb = persist.tile([hd, s], bf16, tag="m1k")
            m2k_sb = persist.tile([hd, s], bf16, tag="m2k")
            tri_sb = const.tile([128, 128], bf16, tag="tri")

            ones_k = const.tile([128, 1], bf16, tag="ones_k")
            nc.vector.memset(ones_k[:], 1.0)
            ones_1 = const.tile([1, 128], bf16, tag="ones_1")
            nc.vector.memset(ones_1[:], 1.0)
            eps_sb = const.tile([1, 1], f32, tag="eps")
            nc.vector.memset(eps_sb[:], EPS)

            # ---- persistent activations ---------------------------------
            qT_sb = [persist.tile([hd, s], bf16, tag=f"qT{h}", name=f"qT{h}")
                     for h in range(nhl)]
            kT_sb = persist.tile([hd, s], bf16, tag="kT")
            v_sb = persist.tile([128, kb_n, hd], bf16, tag="v")
            attT_sb = [persist.tile([hd, s], bf16, tag=f"attT{h}",
                                    name=f"attT{h}") for h in range(nhl)]

            xT_re = xT_d.rearrange("(kc p) n -> p kc n", p=128)

            # PE warmup: dummy matmuls with no input deps so the HAM
            # clock-gate ramps to 8/8 while the first DMAs are in flight.
            warm_rhs = wa.tile([128, 512], bf16, tag="warm_rhs")
            nc.vector.memset(warm_rhs[:], 0.0)
            wps = big.tile([128, 512], f32, tag="big", name="warm")
            for _ in range(56):
                nc.tensor.matmul(wps[:], warm_rhs[:, 0:128], warm_rhs[:])

            def norm_rope_chain(q_ps, t, cs):
                # rmsnorm via ones-matmul + bcast-matmul, rope via coeff
                # tiles with the even/odd half-swap done by partition-offset
                # DVE reads.
                sq = wa.tile([128, 512], bf16, tag="sq", name="sq")
                nc.scalar.activation(sq[:], q_ps[:], AF.Square)
                ssq = ssl.tile([1, 512], f32, tag="ssl", name="ssq")
                nc.tensor.matmul(ssq[:], ones_k[:], sq[:])
                tmp1 = wa.tile([1, 512], f32, tag="tmp1", name="tmp1")
                nc.scalar.activation(
                    tmp1[:], ssq[:], AF.Sqrt, scale=1.0 / hd, bias=eps_sb[:]
                )
                rb1 = wa.tile([1, 512], f32, tag="rb1", name="rb1")
                nc.vector.reciprocal_approx_fast(rb1[:], tmp1[:])
                rb1b = wa.tile([1, 512], bf16, tag="rb1b", name="rb1b")
                nc.vector.tensor_copy(rb1b[:], rb1[:])
                rb_ps = big.tile([128, 512], f32, tag="big", name="rb_ps")
                nc.tensor.matmul(rb_ps[:], ones_1[:], rb1b[:])
                # rope rotation on the unnormalized q (rotation commutes
                # with the per-column rsqrt scale, applied last)
                m1 = m1q_sb if t < nhl else m1k_sb
                m2 = m2q_sb if t < nhl else m2k_sb
                t1 = wa.tile([128, 512], f32, tag="t1", name="t1")
                nc.vector.tensor_mul(t1[:], q_ps[:], m1[:, cs])
                # t2 = swap_halves(q) * m2, via partition-offset reads
                t2 = wa.tile([128, 512], f32, tag="t2", name="t2")
                nc.vector.tensor_mul(t2[0:64, :], q_ps[64:128, :], m2[0:64, cs])
                nc.vector.tensor_mul(t2[64:128, :], q_ps[0:64, :], m2[64:128, cs])
                u = wa.tile([128, 512], f32, tag="u", name="u")
                nc.vector.tensor_add(u[:], t1[:], t2[:])
                dest = qT_sb[t] if t < nhl else kT_sb
                nc.vector.tensor_mul(dest[:, cs], u[:], rb_ps[:])

            # ---------------- o-projection emission helper ----------------
            ysb_tiles = {}

            def emit_oproj_block(qt, i):
                # i-th [128-token x 512-dcol] block of o-projection for
                # q-tile qt (i in 0..15; 4 tt x 4 db, db-major per tt).
                tt = qt * 4 + i // 4
                db = i % 4
                if db == 0:
                    ysb_tiles[tt] = ysbp.tile([128, d], f16, tag="ysb",
                                              name=f"ysb{tt}")
                y_ps = big.tile([128, 512], f32, tag="big", name="y_ps")
                for hh in range(nhl):
                    nc.tensor.matmul(
                        y_ps[:],
                        attT_sb[hh][:, tt * 128:(tt + 1) * 128],
                        wo_sb[:, hh, db * 512:(db + 1) * 512],
                        start=(hh == 0), stop=(hh == nhl - 1),
                    )
                y_sb = ysb_tiles[tt]
                eng = nc.scalar if (i % 2 == 0) else nc.vector
                if eng is nc.scalar:
                    nc.scalar.copy(y_sb[:, db * 512:(db + 1) * 512], y_ps[:])
                else:
                    nc.vector.tensor_copy(
                        y_sb[:, db * 512:(db + 1) * 512], y_ps[:])
                if tt == s // 128 - 1:
                    # final token stripe: per-db DMA so the drain overlaps
                    nc.sync.dma_start(
                        y_d[tt * 128:(tt + 1) * 128,
                            db * 512:(db + 1) * 512],
                        y_sb[:, db * 512:(db + 1) * 512])
                    if db == 3:
                        del ysb_tiles[tt]
                elif db == 3:
                    nc.sync.dma_start(
                        y_d[tt * 128:(tt + 1) * 128, :], y_sb[:])
                    del ysb_tiles[tt]

            # ---------------- attention emission helper -------------------
            def emit_attention(qt, oproj_blocks=()):
                # oproj_blocks: list of (oqt, i) o-projection blocks to
                # interleave, split evenly across the 4 head tails.
                qcs = slice(qt * 512, (qt + 1) * 512)
                nkb = 4 * qt + 4
                per_head = len(oproj_blocks) // nhl

                def emit_tail(attT_ps, l_acc, h):
                    # normalize: attT_sb = attT_ps * bcast(1 / colsum).
                    # broadcast first, reciprocal after (parallel across
                    # partitions — DVE is serial along the free dim).
                    l_ps = ssl.tile([1, 512], f32, tag="ssl", name="l_ps")
                    nc.tensor.matmul(l_ps[:], ones_k[:], l_acc[:])
                    lb = wa.tile([1, 512], bf16, tag="rb1b", name="lb")
                    nc.scalar.copy(lb[:], l_ps[:])
                    lbc_ps = big.tile([128, 512], f32, tag="big", name="lbc")
                    nc.tensor.matmul(lbc_ps[:], ones_1[:], lb[:])
                    recl = wa.tile([128, 512], f32, tag="recl", name="recl")
                    nc.vector.reciprocal_approx_fast(recl[:], lbc_ps[:])
                    nc.vector.tensor_mul(
                        attT_sb[h][:, qcs], attT_ps[:], recl[:])
                    for oqt, i in oproj_blocks[h * per_head:(h + 1) * per_head]:
                        emit_oproj_block(oqt, i)

                pending_tail = None
                for h in range(nhl):
                    attT_ps = attp.tile([128, 512], f32, tag="attp",
                                        name=f"attT{qt}_{h}")
                    l_acc = lac.tile([128, 512], bf16, tag="lacc",
                                     name=f"lacc{qt}_{h}")
                    s_tiles = {}

                    def emit_s(kb):
                        sp = big.tile([128, 512], f32, tag="big", name="s_ps")
                        r = kb - 4 * qt
                        c0 = 128 * r if r > 0 else 0
                        nc.tensor.matmul(
                            sp[:, c0:512],
                            kT_sb[:, kb * 128:(kb + 1) * 128],
                            qT_sb[h][:, qt * 512 + c0:(qt + 1) * 512],
                        )
                        s_tiles[kb] = sp

                    emit_s(0)
                    if nkb > 1:
                        emit_s(1)
                    if pending_tail is not None:
                        emit_tail(*pending_tail)
                        pending_tail = None
                    for kb in range(nkb):
                        if kb + 2 < nkb:
                            emit_s(kb + 2)
                        sp = s_tiles.pop(kb)
                        p = pb.tile([128, 512], bf16, tag="p")
                        r = kb - 4 * qt
                        c0 = 128 * r if r > 0 else 0
                        nc.scalar.activation(
                            p[:, c0:512], sp[:, c0:512], AF.Exp, scale=SCALE,
                        )
                        if r >= 0:
                            nc.vector.tensor_mul(
                                p[:, 128 * r:128 * (r + 1)],
                                p[:, 128 * r:128 * (r + 1)],
                                tri_sb[:],
                            )
                        # denominator accumulation
                        if kb == 0:
                            nc.vector.tensor_copy(l_acc[:], p[:])
                        else:
                            nc.vector.tensor_add(
                                l_acc[:, c0:512], l_acc[:, c0:512],
                                p[:, c0:512],
                            )
                        # transposed PV: stationary v block, stream p.
                        # PSUM start/stop groups are bank-granular: start
                        # only on the first write, stop only on the last.
                        nc.tensor.matmul(
                            attT_ps[:, c0:512], v_sb[:, kb, :], p[:, c0:512],
                            start=(kb == 0), stop=(kb == nkb - 1),
                        )
                    pending_tail = (attT_ps, l_acc, h)
                emit_tail(*pending_tail)

            # ================= main emission ==============================
            xts = {}

            def emit_xt(nb):
                xts[nb] = xtp.tile([128, kc_n, 512], bf16, tag="xt",
                                   name=f"xt{nb}")
                ncs = slice(nb * 512, (nb + 1) * 512)
                for kc in range(kc_n):
                    eng = nc.sync if kc < 8 else nc.scalar
                    eng.dma_start(xts[nb][:, kc, :], xT_re[:, kc, ncs])
                # rope/norm coefficient slices for this block
                nc.sync.dma_start(m1q_sb[:, ncs], m1q_d[:, ncs])
                nc.scalar.dma_start(m2q_sb[:, ncs], m2q_d[:, ncs])
                nc.sync.dma_start(m1k_sb[:, ncs], m1k_d[:, ncs])
                nc.scalar.dma_start(m2k_sb[:, ncs], m2k_d[:, ncs])

            for nb in range(nb_n):
                cs = slice(nb * 512, (nb + 1) * 512)
                if nb == 0:
                    emit_xt(0)
                    # weights, in consumption order, HWDGE queues
                    for kc in range(0, 8):
                        nc.sync.dma_start(wq_sb[:, kc, :], wq_re[:, kc, :])
                    for kc in range(8, 16):
                        nc.scalar.dma_start(wq_sb[:, kc, :], wq_re[:, kc, :])
                    nc.scalar.dma_start(wk_sb[:], wk_re[:])
                    nc.sync.dma_start(tri_sb[:], tri_d[:])
                    nc.gpsimd.dma_start(wv_sb[:], wv_re[:])
                    for h in range(nhl):
                        nc.gpsimd.dma_start(wo_sb[:, h, :], wo_re[:, h, :])
                if nb + 1 < nb_n:
                    emit_xt(nb + 1)
                xt = xts.pop(nb)

                # k first (small weight arrives earliest), then q heads;
                # norm chain deferred one tensor
                pending = None
                for t in [nhl] + list(range(nhl)):
                    q_ps = big.tile([128, 512], f32, tag="big", name="q_ps")
                    for kc in range(kc_n):
                        if t < nhl:
                            lhsT = wq_sb[:, kc, t * hd:(t + 1) * hd]
                        else:
                            lhsT = wk_sb[:, kc, :]
                        nc.tensor.matmul(
                            q_ps[:], lhsT, xt[:, kc, :],
                            start=(kc == 0), stop=(kc == kc_n - 1),
                        )
                    if pending is not None:
                        norm_rope_chain(*pending)
                    pending = (q_ps, t, cs)

                # v: transposed projection (512-wide streams), then DMA
                # XBAR transpose to token-major
                vT_ps = vps.tile([128, 512], f32, tag="vps", name="vT_ps")
                for kc in range(kc_n):
                    nc.tensor.matmul(
                        vT_ps[:], wv_sb[:, kc, :], xt[:, kc, :],
                        start=(kc == 0), stop=(kc == kc_n - 1),
                    )
                norm_rope_chain(*pending)
                pending = None
                vT_sb = wa.tile([128, 512], bf16, tag="vT", name="vT_sb")
                nc.vector.tensor_copy(vT_sb[:], vT_ps[:])
                for tt in range(4):
                    nc.sync.dma_start_transpose(
                        v_sb[:, nb * 4 + tt, :],
                        vT_sb[:, tt * 128:(tt + 1) * 128],
                    )

                if nb >= 1:
                    blocks = [(0, i) for i in range(16)] if nb == 3 else ()
                    emit_attention(nb - 1, blocks)

            # last attention interleaves oproj(1) + first half of oproj(2);
            # the tail then runs the rest (PE-only, fills the drain window)
            emit_attention(
                qt_n - 1,
                [(1, i) for i in range(16)] + [(2, i) for i in range(8)],
            )
            for i in range(8, 16):
                emit_oproj_block(2, i)
            for i in range(16):
                emit_oproj_block(3, i)

    nc.compile()
    return nc


def _rope_coeffs(norm_w, s=S, hd=HD):
    """Coefficient tiles [hd, s] folding rope cos/sin + permuted norm weight."""
    perm = np.concatenate([np.arange(0, hd, 2), np.arange(1, hd, 2)])
    w = np.asarray(norm_w, np.float64)[perm]
    half = hd // 2
    pos = np.arange(s, dtype=np.float64)
    inv_freq = 1.0 / (THETA ** (np.arange(0, hd, 2, dtype=np.float64) / hd))
    ang = pos[None, :] * inv_freq[:, None]          # [half, s]
    cos, sin = np.cos(ang), np.sin(ang)
    m1 = np.empty((hd, s), np.float32)
    m2 = np.empty((hd, s), np.float32)
    m1[:half] = cos * w[:half, None]
    m1[half:] = cos * w[half:, None]
    m2[:half] = -sin * w[half:, None]
    m2[half:] = sin * w[:half, None]
    return m1, m2


def _host_prep(x, wq, wk, wv, wo, q_norm_w, k_norm_w):
    perm = np.concatenate([np.arange(0, HD, 2), np.arange(1, HD, 2)])
    m1q, m2q = _rope_coeffs(q_norm_w)
    m1k, m2k = _rope_coeffs(k_norm_w)
    tri = np.triu(np.ones((128, 128), np.float32)).astype(BF16)

    in_maps = []
    for c in range(8):
        b, g = c // 4, c % 4
        heads = range(NHL * g, NHL * g + NHL)
        wq_loc = np.concatenate(
            [wq[:, h * HD:(h + 1) * HD][:, perm] for h in heads], axis=1
        )
        in_maps.append({
            "xT": np.ascontiguousarray(x[b].T).astype(BF16),
            "wq": np.ascontiguousarray(wq_loc).astype(BF16),
            "wk": np.ascontiguousarray(wk[:, g * HD:(g + 1) * HD][:, perm]).astype(BF16),
            "wv": np.ascontiguousarray(wv[:, g * HD:(g + 1) * HD]).astype(BF16),
            "wo": np.ascontiguousarray(wo[NHL * g * HD:NHL * (g + 1) * HD, :]).astype(BF16),
            "m1q": m1q.astype(BF16), "m2q": m2q.astype(BF16),
            "m1k": m1k.astype(BF16), "m2k": m2k.astype(BF16),
            "tri": tri,
        })
    return in_maps


def _install_ntff_shim():
    import types
    if "antenv.axon_hooks" in sys.modules:
        return
    mod = types.ModuleType("antenv.axon_hooks")
    _hook = [None]
    mod.set_axon_ntff_profile_hook = lambda h: _hook.__setitem__(0, h)
    mod.get_axon_ntff_profile_hook = lambda: _hook[0]
    sys.modules["antenv.axon_hooks"] = mod
    try:
        from trn_agent_boot.trn_boot import _ntff_profile_via_ctypes
        mod.set_axon_ntff_profile_hook(
            _ntff_profile_via_ctypes("/opt/axon/libaxon_pjrt.so")
        )
    except Exception:
        pass


LAST_EXEC_NS = None


def kernel(x, wq, wk, wv, wo, q_norm_w, k_norm_w):
    global LAST_EXEC_NS
    from concourse import bass_utils

    x = np.asarray(x)
    if "nc" not in _CACHED:
        _CACHED["nc"] = build_nc()
    nc = _CACHED["nc"]

    in_maps = _host_prep(
        np.asarray(x, np.float32), np.asarray(wq, np.float32),
        np.asarray(wk, np.float32), np.asarray(wv, np.float32),
        np.asarray(wo, np.float32), np.asarray(q_norm_w, np.float32),
        np.asarray(k_norm_w, np.float32),
    )
    trace = bool(int(os.environ.get("BASS_KERNEL_TRACE", "0")))
    if trace:
        _install_ntff_shim()
    res = bass_utils.run_bass_kernel_spmd(
        nc, in_maps, core_ids=list(range(8)), trace=trace
    )
    LAST_EXEC_NS = res.exec_time_ns
    y = np.zeros((B, S, D), np.float32)
    for c in range(8):
        y[c // 4] += res.results[c]["y"]
    return y


# revision 25
# speedup vs baseline: 1.1629x; 1.1629x over previous
"""Trainium2 Bass kernel for GQA attention block (nn_Attention_81372450390110).

Module: y = AttnOut(x) with q/k RMSNorm + interleaved RoPE + causal GQA
(NH=16 q heads, KVH=4 kv heads, HD=128, D=2048, B=2, S=2048).

Sharding: 8 cores = 2 batches x 4 KV groups. Core c handles batch c//4 and
KV group c%4 (4 q heads + 1 kv head). Each core computes a full [S, D]
partial of the output projection (row-parallel over heads); the host sums
the 4 group-partials per batch (fp16 partials, fp32 accumulate).

v2 layout strategy (vs v1):
  - transposed PV: stationary = v block (reused across heads/q-tiles),
    streaming p columns -> attT [hd, q] lands directly in PSUM; kills the
    LDWEIGHTS-bound 129-col PV matmuls and all PE transposes
  - softmax denominator l via DVE bf16 accumulation of p tiles + one
    ones-matmul per (head, q-tile) + ones-broadcast matmul for 1/l
  - phase interleaving: attention for q-tile qt emitted right after
    projection block nb=qt+1; o-projection of qt interleaved into
    attention of qt+2; single shared PSUM ring
  - V computed transposed (512-wide streams) then moved to token-major
    via DMA XBAR transpose (free wrt engines)
  - rope half-swap via DVE partition-offset reads (no SBUF-SBUF DMA)
  - DMA schedule: latency-critical loads on the two HWDGE queues
    (sync+scalar) in consumption order; wv/wo on the gpsimd SW queue
  - y output in fp16, written per 128-token stripe
"""

import os
import sys

sys.path.insert(0, "/opt/trn_rl_repo")

import numpy as np
import ml_dtypes

BF16 = ml_dtypes.bfloat16

B = 2
S = 2048
D = 2048
NH = 16
KVH = 4
HD = 128
THETA = 10000.0
EPS = 1e-6
NHL = NH // KVH  # q heads per core (4)
SCALE = 1.0 / float(np.sqrt(HD))

_CACHED = {}


def build_nc(s=S, d=D, nhl=NHL, hd=HD):
    import concourse.mybir as mybir
    import concourse.tile as tile
    from concourse import bacc

    f32 = mybir.dt.float32
    f16 = mybir.dt.float16
    bf16 = mybir.dt.bfloat16
    AF = mybir.ActivationFunctionType

    kc_n = d // 128          # contraction chunks for projections
    nb_n = s // 512          # 512-token blocks
    qt_n = s // 512          # q tiles (512 wide) in attention
    kb_n = s // 128          # k blocks (128 wide)

    nc = bacc.Bacc("TRN2", target_bir_lowering=False, debug=False)

    xT_d = nc.dram_tensor("xT", (d, s), bf16, kind="ExternalInput")
    wq_d = nc.dram_tensor("wq", (d, nhl * hd), bf16, kind="ExternalInput")
    wk_d = nc.dram_tensor("wk", (d, hd), bf16, kind="ExternalInput")
    wv_d = nc.dram_tensor("wv", (d, hd), bf16, kind="ExternalInput")
    wo_d = nc.dram_tensor("wo", (nhl * hd, d), bf16, kind="ExternalInput")
    m1q_d = nc.dram_tensor("m1q", (hd, s), bf16, kind="ExternalInput")
    m2q_d = nc.dram_tensor("m2q", (hd, s), bf16, kind="ExternalInput")
    m1k_d = nc.dram_tensor("m1k", (hd, s), bf16, kind="ExternalInput")
    m2k_d = nc.dram_tensor("m2k", (hd, s), bf16, kind="ExternalInput")
    tri_d = nc.dram_tensor("tri", (128, 128), bf16, kind="ExternalInput")
    y_d = nc.dram_tensor("y", (s, d), f16, kind="ExternalOutput")

    with tile.TileContext(nc) as tc, nc.allow_low_precision(
        reason="bf16 compute by design; fp32 accumulation in PSUM"
    ):
        with (
            tc.tile_pool(name="const", bufs=1) as const,
            tc.tile_pool(name="persist", bufs=1) as persist,
            tc.tile_pool(name="xtp", bufs=2) as xtp,
            tc.tile_pool(name="wa", bufs=3) as wa,
            tc.tile_pool(name="pb", bufs=4) as pb,
            tc.tile_pool(name="lac", bufs=2) as lac,
            tc.tile_pool(name="ysb", bufs=2) as ysbp,
            tc.tile_pool(name="big", bufs=4, space="PSUM") as big,
            tc.tile_pool(name="attp", bufs=2, space="PSUM") as attp,
            tc.tile_pool(name="vps", bufs=1, space="PSUM") as vps,
            tc.tile_pool(name="ssl", bufs=1, space="PSUM") as ssl,
        ):
            # ---- resident weights / coefficients -------------------------
            wq_sb = persist.tile([128, kc_n, nhl * hd], bf16, tag="wq")
            wq_re = wq_d.rearrange("(kc p) m -> p kc m", p=128)
            wk_sb = persist.tile([128, kc_n, hd], bf16, tag="wk")
            wk_re = wk_d.rearrange("(kc p) m -> p kc m", p=128)
            wv_sb = persist.tile([128, kc_n, hd], bf16, tag="wv")
            wv_re = wv_d.rearrange("(kc p) m -> p kc m", p=128)
            wo_sb = persist.tile([128, nhl, d], bf16, tag="wo")
            wo_re = wo_d.rearrange("(h p) m -> p h m", p=128)

            m1q_sb = persist.tile([hd, s], bf16, tag="m1q")
            m2q_sb = persist.tile([hd, s], bf16, tag="m2q")
            m1k_sb = persist.tile([hd, s], bf16, tag="m1k")
            m2k_sb = persist.tile([hd, s], bf16, tag="m2k")
            tri_sb = const.tile([128, 128], bf16, tag="tri")

            ones_k = const.tile([128, 1], bf16, tag="ones_k")
            nc.vector.memset(ones_k[:], 1.0)
            ones_1 = const.tile([1, 128], bf16, tag="ones_1")
            nc.vector.memset(ones_1[:], 1.0)
            eps_sb = const.tile([1, 1], f32, tag="eps")
            nc.vector.memset(eps_sb[:], EPS)

            # ---- persistent activations ---------------------------------
            qT_sb = [persist.tile([hd, s], bf16, tag=f"qT{h}", name=f"qT{h}")
                     for h in range(nhl)]
            kT_sb = persist.tile([hd, s], bf16, tag="kT")
            v_sb = persist.tile([128, kb_n, hd], bf16, tag="v")
            attT_sb = [persist.tile([hd, s], bf16, tag=f"attT{h}",
                                    name=f"attT{h}") for h in range(nhl)]

            xT_re = xT_d.rearrange("(kc p) n -> p kc n", p=128)

            # PE warmup: dummy matmuls with no input deps so the HAM
            # clock-gate ramps to 8/8 while the first DMAs are in flight.
            warm_rhs = wa.tile([128, 512], bf16, tag="warm_rhs")
            nc.vector.memset(warm_rhs[:], 0.0)
            wps = big.tile([128, 512], f32, tag="big", name="warm")
            for _ in range(24):
                nc.tensor.matmul(wps[:], warm_rhs[:, 0:128], warm_rhs[:])

            def norm_rope_chain(q_ps, t, cs):
                # rmsnorm via ones-matmul + bcast-matmul, rope via coeff
                # tiles with the even/odd half-swap done by partition-offset
                # DVE reads.
                sq = wa.tile([128, 512], bf16, tag="sq", name="sq")
                nc.scalar.activation(sq[:], q_ps[:], AF.Square)
                ssq = ssl.tile([1, 512], f32, tag="ssl", name="ssq")
                nc.tensor.matmul(ssq[:], ones_k[:], sq[:])
                tmp1 = wa.tile([1, 512], f32, tag="tmp1", name="tmp1")
                nc.scalar.activation(
                    tmp1[:], ssq[:], AF.Sqrt, scale=1.0 / hd, bias=eps_sb[:]
                )
                rb1 = wa.tile([1, 512], f32, tag="rb1", name="rb1")
                nc.vector.reciprocal_approx_fast(rb1[:], tmp1[:])
                rb1b = wa.tile([1, 512], bf16, tag="rb1b", name="rb1b")
                nc.vector.tensor_copy(rb1b[:], rb1[:])
                rb_ps = big.tile([128, 512], f32, tag="big", name="rb_ps")
                nc.tensor.matmul(rb_ps[:], ones_1[:], rb1b[:])
                # rope rotation on the unnormalized q (rotation commutes
                # with the per-column rsqrt scale, applied last)
                m1 = m1q_sb if t < nhl else m1k_sb
                m2 = m2q_sb if t < nhl else m2k_sb
                t1 = wa.tile([128, 512], f32, tag="t1", name="t1")
                nc.vector.tensor_mul(t1[:], q_ps[:], m1[:, cs])
                # t2 = swap_halves(q) * m2, via partition-offset reads
                t2 = wa.tile([128, 512], f32, tag="t2", name="t2")
                nc.vector.tensor_mul(t2[0:64, :], q_ps[64:128, :], m2[0:64, cs])
                nc.vector.tensor_mul(t2[64:128, :], q_ps[0:64, :], m2[64:128, cs])
                u = wa.tile([128, 512], f32, tag="u", name="u")
                nc.vector.tensor_add(u[:], t1[:], t2[:])
                dest = qT_sb[t] if t < nhl else kT_sb
                nc.vector.tensor_mul(dest[:, cs], u[:], rb_ps[:])

            # ---------------- o-projection emission helper ----------------
            ysb_tiles = {}

            def emit_oproj_block(qt, i):
                # i-th [128-token x 512-dcol] block of o-projection for
                # q-tile qt (i in 0..15; 4 tt x 4 db, db-major per tt).
                tt = qt * 4 + i // 4
                db = i % 4
                if db == 0:
                    ysb_tiles[tt] = ysbp.tile([128, d], f16, tag="ysb",
                                              name=f"ysb{tt}")
                y_ps = big.tile([128, 512], f32, tag="big", name="y_ps")
                for hh in range(nhl):
                    nc.tensor.matmul(
                        y_ps[:],
                        attT_sb[hh][:, tt * 128:(tt + 1) * 128],
                        wo_sb[:, hh, db * 512:(db + 1) * 512],
                        start=(hh == 0), stop=(hh == nhl - 1),
                    )
                y_sb = ysb_tiles[tt]
                eng = nc.scalar if (i % 2 == 0) else nc.vector
                if eng is nc.scalar:
                    nc.scalar.copy(y_sb[:, db * 512:(db + 1) * 512], y_ps[:])
                else:
                    nc.vector.tensor_copy(
                        y_sb[:, db * 512:(db + 1) * 512], y_ps[:])
                if tt == s // 128 - 1:
                    # final token stripe: per-db DMA so the drain overlaps
                    nc.sync.dma_start(
                        y_d[tt * 128:(tt + 1) * 128,
                            db * 512:(db + 1) * 512],
                        y_sb[:, db * 512:(db + 1) * 512])
                    if db == 3:
                        del ysb_tiles[tt]
                elif db == 3:
                    nc.sync.dma_start(
                        y_d[tt * 128:(tt + 1) * 128, :], y_sb[:])
                    del ysb_tiles[tt]

            # ---------------- attention emission helper -------------------
            def emit_attention(qt, oproj_blocks=()):
                # oproj_blocks: list of (oqt, i) o-projection blocks to
                # interleave, split evenly across the 4 head tails.
                qcs = slice(qt * 512, (qt + 1) * 512)
                nkb = 4 * qt + 4
                per_head = len(oproj_blocks) // nhl

                def emit_tail(attT_ps, l_acc, h):
                    # normalize: attT_sb = attT_ps * bcast(1 / colsum).
                    # broadcast first, reciprocal after (parallel across
                    # partitions — DVE is serial along the free dim).
                    l_ps = ssl.tile([1, 512], f32, tag="ssl", name="l_ps")
                    nc.tensor.matmul(l_ps[:], ones_k[:], l_acc[:])
                    lb = wa.tile([1, 512], bf16, tag="rb1b", name="lb")
                    nc.scalar.copy(lb[:], l_ps[:])
                    lbc_ps = big.tile([128, 512], f32, tag="big", name="lbc")
                    nc.tensor.matmul(lbc_ps[:], ones_1[:], lb[:])
                    recl = wa.tile([128, 512], f32, tag="recl", name="recl")
                    nc.vector.reciprocal_approx_fast(recl[:], lbc_ps[:])
                    nc.vector.tensor_mul(
                        attT_sb[h][:, qcs], attT_ps[:], recl[:])
                    for oqt, i in oproj_blocks[h * per_head:(h + 1) * per_head]:
                        emit_oproj_block(oqt, i)

                pending_tail = None
                for h in range(nhl):
                    attT_ps = attp.tile([128, 512], f32, tag="attp",
                                        name=f"attT{qt}_{h}")
                    l_acc = lac.tile([128, 512], bf16, tag="lacc",
                                     name=f"lacc{qt}_{h}")
                    s_tiles = {}

                    def emit_s(kb):
                        sp = big.tile([128, 512], f32, tag="big", name="s_ps")
                        r = kb - 4 * qt
                        c0 = 128 * r if r > 0 else 0
                        nc.tensor.matmul(
                            sp[:, c0:512],
                            kT_sb[:, kb * 128:(kb + 1) * 128],
                            qT_sb[h][:, qt * 512 + c0:(qt + 1) * 512],
                        )
                        s_tiles[kb] = sp

                    emit_s(0)
                    if nkb > 1:
                        emit_s(1)
                    if pending_tail is not None:
                        emit_tail(*pending_tail)
                        pending_tail = None
                    for kb in range(nkb):
                        if kb + 2 < nkb:
                            emit_s(kb + 2)
                        sp = s_tiles.pop(kb)
                        p = pb.tile([128, 512], bf16, tag="p")
                        r = kb - 4 * qt
                        c0 = 128 * r if r > 0 else 0
                        nc.scalar.activation(
                            p[:, c0:512], sp[:, c0:512], AF.Exp, scale=SCALE,
                        )
                        if r >= 0:
                            nc.vector.tensor_mul(
                                p[:, 128 * r:128 * (r + 1)],
                                p[:, 128 * r:128 * (r + 1)],
                                tri_sb[:],
                            )
                        # denominator accumulation
                        if kb == 0:
                            nc.vector.tensor_copy(l_acc[:], p[:])
                        else:
                            nc.vector.tensor_add(
                                l_acc[:, c0:512], l_acc[:, c0:512],
                                p[:, c0:512],
                            )
                        # transposed PV: stationary v block, stream p.
                        # PSUM start/stop groups are bank-granular: start
                        # only on the first write, stop only on the last.
                        nc.tensor.matmul(
                            attT_ps[:, c0:512], v_sb[:, kb, :], p[:, c0:512],
                            start=(kb == 0), stop=(kb == nkb - 1),
                        )
                    pending_tail = (attT_ps, l_acc, h)
                emit_tail(*pending_tail)

            # ================= main emission ==============================
            xts = {}

            def emit_xt(nb):
                xts[nb] = xtp.tile([128, kc_n, 512], bf16, tag="xt",
                                   name=f"xt{nb}")
                ncs = slice(nb * 512, (nb + 1) * 512)
                for kc in range(kc_n):
                    eng = nc.sync if kc < 8 else nc.scalar
                    eng.dma_start(xts[nb][:, kc, :], xT_re[:, kc, ncs])
                # rope/norm coefficient slices for this block
                nc.sync.dma_start(m1q_sb[:, ncs], m1q_d[:, ncs])
                nc.scalar.dma_start(m2q_sb[:, ncs], m2q_d[:, ncs])
                nc.sync.dma_start(m1k_sb[:, ncs], m1k_d[:, ncs])
                nc.scalar.dma_start(m2k_sb[:, ncs], m2k_d[:, ncs])

            for nb in range(nb_n):
                cs = slice(nb * 512, (nb + 1) * 512)
                if nb == 0:
                    emit_xt(0)
                    # weights, in consumption order, HWDGE queues
                    for kc in range(0, 8):
                        nc.sync.dma_start(wq_sb[:, kc, :], wq_re[:, kc, :])
                    for kc in range(8, 16):
                        nc.scalar.dma_start(wq_sb[:, kc, :], wq_re[:, kc, :])
                    nc.scalar.dma_start(wk_sb[:], wk_re[:])
                    nc.sync.dma_start(tri_sb[:], tri_d[:])
                    nc.gpsimd.dma_start(wv_sb[:], wv_re[:])
                    for h in range(nhl):
                        nc.gpsimd.dma_start(wo_sb[:, h, :], wo_re[:, h, :])
                if nb + 1 < nb_n:
                    emit_xt(nb + 1)
                xt = xts.pop(nb)

                # q heads then k: projection MMs now, norm chain deferred
                pending = None
                for t in list(range(nhl)) + [nhl]:
                    q_ps = big.tile([128, 512], f32, tag="big", name="q_ps")
                    for kc in range(kc_n):
                        if t < nhl:
                            lhsT = wq_sb[:, kc, t * hd:(t + 1) * hd]
                        else:
                            lhsT = wk_sb[:, kc, :]
                        nc.tensor.matmul(
                            q_ps[:], lhsT, xt[:, kc, :],
                            start=(kc == 0), stop=(kc == kc_n - 1),
                        )
                    if pending is not None:
                        norm_rope_chain(*pending)
                    pending = (q_ps, t, cs)

                # v: transposed projection (512-wide streams), then DMA
                # XBAR transpose to token-major
                vT_ps = vps.tile([128, 512], f32, tag="vps", name="vT_ps")
                for kc in range(kc_n):
                    nc.tensor.matmul(
                        vT_ps[:], wv_sb[:, kc, :], xt[:, kc, :],
                        start=(kc == 0), stop=(kc == kc_n - 1),
                    )
                norm_rope_chain(*pending)
                pending = None
                vT_sb = wa.tile([128, 512], bf16, tag="vT", name="vT_sb")
                nc.vector.tensor_copy(vT_sb[:], vT_ps[:])
                for tt in range(4):
                    nc.sync.dma_start_transpose(
                        v_sb[:, nb * 4 + tt, :],
                        vT_sb[:, tt * 128:(tt + 1) * 128],
                    )

                if nb >= 1:
                    blocks = [(0, i) for i in range(16)] if nb == 3 else ()
                    emit_attention(nb - 1, blocks)

            # last attention interleaves oproj(1) + first half of oproj(2);
            # the tail then runs the rest (PE-only, fills the drain window)
            emit_attention(
                qt_n - 1,
                [(1, i) for i in range(16)] + [(2, i) for i in range(8)],
            )
            for i in range(8, 16):
                emit_oproj_block(2, i)
            for i in range(16):
                emit_oproj_block(3, i)

    nc.compile()
    return nc


def _rope_coeffs(norm_w, s=S, hd=HD):
    """Coefficient tiles [hd, s] folding rope cos/sin + permuted norm weight."""
    perm = np.concatenate([np.arange(0, hd, 2), np.arange(1, hd, 2)])
    w = np.asarray(norm_w, np.float64)[perm]
    half = hd // 2
    pos = np.arange(s, dtype=np.float64)
    inv_freq = 1.0 / (THETA ** (np.arange(0, hd, 2, dtype=np.float64) / hd))
    ang = pos[None, :] * inv_freq[:, None]          # [half, s]
    cos, sin = np.cos(ang), np.sin(ang)
    m1 = np.empty((hd, s), np.float32)
    m2 = np.empty((hd, s), np.float32)
    m1[:half] = cos * w[:half, None]
    m1[half:] = cos * w[half:, None]
    m2[:half] = -sin * w[half:, None]
    m2[half:] = sin * w[:half, None]
    return m1, m2


def _host_prep(x, wq, wk, wv, wo, q_norm_w, k_norm_w):
    perm = np.concatenate([np.arange(0, HD, 2), np.arange(1, HD, 2)])
    m1q, m2q = _rope_coeffs(q_norm_w)
    m1k, m2k = _rope_coeffs(k_norm_w)
    tri = np.triu(np.ones((128, 128), np.float32)).astype(BF16)

    in_maps = []
    for c in range(8):
        b, g = c // 4, c % 4
        heads = range(NHL * g, NHL * g + NHL)
        wq_loc = np.concatenate(
            [wq[:, h * HD:(h + 1) * HD][:, perm] for h in heads], axis=1
        )
        in_maps.append({
            "xT": np.ascontiguousarray(x[b].T).astype(BF16),
            "wq": np.ascontiguousarray(wq_loc).astype(BF16),
            "wk": np.ascontiguousarray(wk[:, g * HD:(g + 1) * HD][:, perm]).astype(BF16),
            "wv": np.ascontiguousarray(wv[:, g * HD:(g + 1) * HD]).astype(BF16),
            "wo": np.ascontiguousarray(wo[NHL * g * HD:NHL * (g + 1) * HD, :]).astype(BF16),
            "m1q": m1q.astype(BF16), "m2q": m2q.astype(BF16),
            "m1k": m1k.astype(BF16), "m2k": m2k.astype(BF16),
            "tri": tri,
        })
    return in_maps


def _install_ntff_shim():
    import types
    if "antenv.axon_hooks" in sys.modules:
        return
    mod = types.ModuleType("antenv.axon_hooks")
    _hook = [None]
    mod.set_axon_ntff_profile_hook = lambda h: _hook.__setitem__(0, h)
    mod.get_axon_ntff_profile_hook = lambda: _hook[0]
    sys.modules["antenv.axon_hooks"] = mod
    try:
        from trn_agent_boot.trn_boot import _ntff_profile_via_ctypes
        mod.set_axon_ntff_profile_hook(
            _ntff_profile_via_ctypes("/opt/axon/libaxon_pjrt.so")
        )
    except Exception:
        pass


LAST_EXEC_NS = None


def kernel(x, wq, wk, wv, wo, q_norm_w, k_norm_w):
    global LAST_EXEC_NS
    from concourse import bass_utils

    x = np.asarray(x)
    if "nc" not in _CACHED:
        _CACHED["nc"] = build_nc()
    nc = _CACHED["nc"]

    in_maps = _host_prep(
        np.asarray(x, np.float32), np.asarray(wq, np.float32),
        np.asarray(wk, np.float32), np.asarray(wv, np.float32),
        np.asarray(wo, np.float32), np.asarray(q_norm_w, np.float32),
        np.asarray(k_norm_w, np.float32),
    )
    trace = bool(int(os.environ.get("BASS_KERNEL_TRACE", "0")))
    if trace:
        _install_ntff_shim()
    res = bass_utils.run_bass_kernel_spmd(
        nc, in_maps, core_ids=list(range(8)), trace=trace
    )
    LAST_EXEC_NS = res.exec_time_ns
    y = np.zeros((B, S, D), np.float32)
    for c in range(8):
        y[c // 4] += res.results[c]["y"]
    return y
